# revision 1
# baseline (speedup 1.0000x reference)
"""MoE FFN (8 experts, top-2) on 8 Trainium2 NeuronCores.

Strategy: expert parallelism with host-side token routing.
  - Host computes the (tiny) gate: logits = x @ gate_w.T, top-2, softmax.
  - Tokens are gathered per expert and padded to a common capacity C.
  - Core e runs a dense FFN (gelu(x@W1[e].T+b1[e])@W2[e].T+b2[e]) over the
    C tokens routed to expert e, all in one SPMD Bass program.
  - Host scatters y back with the combine weights and sums the two
    expert contributions per token.

Device kernel layout (per core):
  FFN1: psum[inter128, tok] += W1T[k*128:, m*128:].T @ xT[k*128:, tok]
        h = gelu(psum + b1)           (ACT, writes bf16)
  FFN2: psum[hid128, tok]  += W2T[k*128:, m*128:].T @ h[k*128:, tok]
        y = psum + b2                 (DVE, writes f32)
Weights held resident in SBUF as bf16; tokens stream in tiles of <=512.
"""

import sys
import types

import numpy as np
import ml_dtypes

import concourse.bass as bass
import concourse.tile as tile
from concourse import mybir
from concourse.bass_utils import run_bass_kernel_spmd
from bass_rust import ScopedClock, VectorClock


def _ensure_axon_hooks():
    """run_bass_kernel_spmd(trace=True) under axon imports antenv.axon_hooks,
    which this image's antenv lacks.  Register an equivalent module backed by
    trn_agent_boot's ctypes NTFF hook so tracing works (and trace=False paths
    are unaffected)."""
    try:
        import antenv.axon_hooks  # noqa: F401
        return
    except ImportError:
        pass
    hook = None
    try:
        from trn_agent_boot.trn_boot import _ntff_profile_via_ctypes
        hook = _ntff_profile_via_ctypes("/opt/axon/libaxon_pjrt.so")
    except Exception:
        hook = None
    mod = types.ModuleType("antenv.axon_hooks")
    _state = {"hook": hook}
    mod.get_axon_ntff_profile_hook = lambda: _state["hook"]
    mod.set_axon_ntff_profile_hook = lambda h: _state.__setitem__("hook", h)
    sys.modules["antenv.axon_hooks"] = mod
    try:
        import antenv
        antenv.axon_hooks = mod
    except ImportError:
        pass


_ensure_axon_hooks()

H = 1024          # hidden
I = 4096          # intermediate
E = 8             # experts
NCORES = 8
BF16 = mybir.dt.bfloat16
F32 = mybir.dt.float32


class _TC(tile.TileContext):
    """TileContext whose tail drain splits its sem waits across SP nops.

    The walrus pinned in this container rejects a Drain instruction carrying
    more than a couple of sync waits ("Too many sync wait commands",
    CoreV3GenImpl.cpp:104).  Emit one wait-carrier nop per logical processor
    instead, then a waitless drain.
    """

    def _drain_and_barrier(self, tick_clock, wait_clock):
        nc = self.nc
        gc = tick_clock.global_clock
        ticks = eval(repr(gc).replace("VectorClock(", "").rstrip(")"))
        for i, t in enumerate(ticks):
            if t > 0:
                partial = [0] * len(ticks)
                partial[i] = t
                carrier = nc.sync.nop(nofuse=True, hint=f"drain_wait_{i}")
                wait_clock.add_sem_waits(
                    carrier.ins, ScopedClock({None: VectorClock(partial)})
                )
        nc.sync.drain()
        nc.all_engine_barrier()
        assert self.sems is not None
        popped = nc._tile_sem_poison_stack.pop()
        assert popped is self._sem_poison
        nc.clear_and_free_semaphores(list(self.sems.allocated().values()))
        nc.all_engine_barrier()


def _split_waits(nc, maxw=1):
    """The pinned walrus rejects instructions carrying more than one
    embedded sync wait ("Too many sync wait commands").  Hoist excess waits
    onto freshly inserted same-engine nops placed directly before the
    instruction — the engine sequencer executes them in order, so the
    semantics are identical."""
    for fn in nc.m.functions:
        for bb in fn.blocks:
            new = []
            changed = False
            for inst in bb.instructions:
                si = inst.sync_info
                waits = list(si.on_wait) if si is not None else []
                if len(waits) > maxw:
                    changed = True
                    n_extra = len(waits) - maxw
                    for i in range(0, n_extra, maxw):
                        nop = mybir.InstNoOp(
                            name=nc.get_next_instruction_name(),
                            engine=inst.engine,
                            sync_info=mybir.SyncInfo(
                                on_wait=waits[i:i + maxw], on_update=[]
                            ),
                            bass_nofuse=True,
                        )
                        nc.register_instruction(nop, overwrite=True)
                        new.append(nop)
                    si.on_wait = waits[n_extra:]
                new.append(inst)
            if changed:
                bb.instructions = new


def _token_tiles(C):
    # Remainder tile last: the first (full) tile's FFN1 masks the W2 load.
    tiles = [512] * (C // 512)
    if C % 512:
        tiles.append(C % 512)
    return tiles


def _build(C):
    """Dense per-expert FFN over C tokens; one SPMD program for all cores."""
    KH = H // 128   # 8  k-tiles over hidden
    KI = I // 128   # 32 k-tiles over inter
    nc = bass.Bass()
    xt = nc.declare_dram_parameter("xt", [H, C], BF16, isOutput=False)
    w1t = nc.declare_dram_parameter("w1t", [H, I], BF16, isOutput=False)
    w2t = nc.declare_dram_parameter("w2t", [I, H], BF16, isOutput=False)
    b1 = nc.declare_dram_parameter("b1", [128, KI], F32, isOutput=False)
    b2 = nc.declare_dram_parameter("b2", [128, KH], F32, isOutput=False)
    yt = nc.declare_dram_parameter("yt", [H, C], F32, isOutput=True)

    with _TC(nc) as tc:
        with (
            tc.tile_pool(name="weights", bufs=1) as wpool,
            tc.tile_pool(name="bias", bufs=1) as bpool,
            tc.tile_pool(name="x", bufs=3) as xpool,
            tc.tile_pool(name="h", bufs=1) as hpool,
            tc.tile_pool(name="o", bufs=4) as opool,
            tc.tile_pool(name="ps1", bufs=4, space="PSUM") as ps1pool,
            tc.tile_pool(name="ps2", bufs=4, space="PSUM") as ps2pool,
        ):
            # Latency-critical small loads on GpSimd SWDGE queues so they
            # don't queue behind the 16 MB of weight traffic on the sync
            # HWDGE queues.
            b1s = bpool.tile([128, KI], F32, tag="b1")
            nc.gpsimd.dma_start(b1s[:], b1[:])
            b2s = bpool.tile([128, KH], F32, tag="b2")
            nc.gpsimd.dma_start(b2s[:], b2[:])
            # W1 in column phases: phase 0 covers the first m-blocks of all
            # k-tiles, so FFN1 can start after ~2 MB instead of ~8 MB.
            w1s = [
                wpool.tile([128, I], BF16, tag=f"w1_{k}", name=f"w1_{k}")
                for k in range(KH)
            ]
            # Small first phase (256 cols = 512 KB) so the first FFN1
            # psum-groups unblock early, then coarse 960-col phases.
            # (Finer 128-col phases measured WORSE: 64 small descriptors
            # slow the aggregate delivery and triple the startup stalls.)
            bounds = [0, 256] + [256 + 960 * i for i in range(1, 5)]
            for lo, hi in zip(bounds[:-1], bounds[1:]):
                for k in range(KH):
                    nc.sync.dma_start(
                        w1s[k][:, lo:hi], w1t[k * 128:(k + 1) * 128, lo:hi]
                    )
            # W2 afterwards, in FFN2 consumption order (k ascending).
            w2s = []
            for k in range(KI):
                w = wpool.tile([128, H], BF16, tag=f"w2_{k}")
                nc.sync.dma_start(w[:], w2t[k * 128:(k + 1) * 128, :])
                w2s.append(w)

            off = 0
            for ti, tw in enumerate(_token_tiles(C)):
                xs = xpool.tile([128, KH * tw], BF16, tag="xt")
                # First tile: halve each chunk so the 8 SWDGE queues turn
                # around faster and the first psum-group unblocks sooner.
                nsplit = 2 if ti == 0 else 1
                for k in range(KH):
                    step = tw // nsplit
                    for s in range(nsplit):
                        nc.gpsimd.dma_start(
                            xs[:, k * tw + s * step:k * tw + (s + 1) * step],
                            xt[k * 128:(k + 1) * 128,
                               off + s * step:off + (s + 1) * step],
                        )
                ht = hpool.tile([128, KI * tw], BF16, tag="h")
                for m in range(KI):
                    ps = ps1pool.tile([128, tw], F32, tag="ps1")
                    for k in range(KH):
                        nc.tensor.matmul(
                            ps[:],
                            w1s[k][:, m * 128:(m + 1) * 128],
                            xs[:, k * tw:(k + 1) * tw],
                            start=(k == 0),
                            stop=(k == KH - 1),
                        )
                    nc.scalar.activation(
                        ht[:, m * tw:(m + 1) * tw],
                        ps[:],
                        mybir.ActivationFunctionType.Gelu,
                        bias=b1s[:, m:m + 1],
                    )
                for m in range(KH):
                    ps = ps2pool.tile([128, tw], F32, tag="ps2")
                    for k in range(KI):
                        nc.tensor.matmul(
                            ps[:],
                            w2s[k][:, m * 128:(m + 1) * 128],
                            ht[:, k * tw:(k + 1) * tw],
                            start=(k == 0),
                            stop=(k == KI - 1),
                        )
                    ot = opool.tile([128, tw], F32, tag="o")
                    nc.vector.tensor_scalar_add(ot[:], ps[:], b2s[:, m:m + 1])
                    nc.scalar.dma_start(
                        yt[m * 128:(m + 1) * 128, off:off + tw], ot[:]
                    )
                off += tw
    _split_waits(nc)
    return nc


def _route(x, gate_w):
    """Host gate: top-2 of 8 logits + softmax over the selected pair."""
    logits = x @ gate_w.T                         # [T, E] f32
    T = logits.shape[0]
    rows = np.arange(T)
    i1 = np.argmax(logits, axis=1)
    v1 = logits[rows, i1]
    masked = logits.copy()
    masked[rows, i1] = -np.inf
    i2 = np.argmax(masked, axis=1)
    v2 = masked[rows, i2]
    # softmax over (v1, v2) with v1 >= v2
    e2 = np.exp(v2 - v1)
    w1 = 1.0 / (1.0 + e2)
    w2 = 1.0 - w1
    return i1, i2, w1.astype(np.float32), w2.astype(np.float32)


def _run(inputs, trace=False):
    hidden_states = np.asarray(inputs["hidden_states"], dtype=np.float32)
    gate_w = np.asarray(inputs["gate_w"], dtype=np.float32)
    W1 = np.asarray(inputs["W1"], dtype=np.float32)
    b1 = np.asarray(inputs["b1"], dtype=np.float32)
    W2 = np.asarray(inputs["W2"], dtype=np.float32)
    b2 = np.asarray(inputs["b2"], dtype=np.float32)

    B, S, _ = hidden_states.shape
    T = B * S
    x = np.ascontiguousarray(hidden_states.reshape(T, H))

    i1, i2, w1, w2 = _route(x, gate_w)
    toks = [np.flatnonzero((i1 == e) | (i2 == e)) for e in range(E)]
    cnts = [len(t) for t in toks]
    C = max(128, -(-max(cnts) // 128) * 128)

    nc = _build(C)

    in_maps = []
    for e in range(E):
        xe = np.zeros((C, H), dtype=ml_dtypes.bfloat16)
        xe[: cnts[e]] = x[toks[e]].astype(ml_dtypes.bfloat16)
        in_maps.append(
            {
                "xt": np.ascontiguousarray(xe.T),
                "w1t": np.ascontiguousarray(W1[e].astype(ml_dtypes.bfloat16).T),
                "w2t": np.ascontiguousarray(W2[e].astype(ml_dtypes.bfloat16).T),
                "b1": np.ascontiguousarray(b1[e].reshape(I // 128, 128).T),
                "b2": np.ascontiguousarray(b2[e].reshape(H // 128, 128).T),
            }
        )

    res = run_bass_kernel_spmd(
        nc, in_maps, core_ids=list(range(NCORES)), trace=trace
    )

    out = np.zeros((T, H), dtype=np.float32)
    for e in range(E):
        te = toks[e]
        ye = res.results[e]["yt"][:, : cnts[e]].T          # [cnt, H]
        we = np.where(i1[te] == e, w1[te], w2[te])
        out[te] += we[:, None] * ye
    return out.reshape(B, S, H), res


def kernel(**inputs):
    out, _ = _run(inputs, trace=False)
    return out



# revision 3
# speedup vs baseline: 1.0174x; 1.0174x over previous
"""MoE FFN (8 experts, top-2) on 8 Trainium2 NeuronCores.

Strategy: balanced expert parallelism with host-side token routing.
  - Host computes the (tiny) gate: logits = x @ gate_w.T, top-2, softmax.
  - Token->expert pairs are balanced across cores in TWO segments:
      segment A: up to A tokens of the core's "primary" expert
      segment B: up to B tokens of a (possibly different) "spill" expert
    (A, B) are chosen so the 8 expert counts pack exactly into 8 A-slots
    + 8 B-slots, minimizing per-core PE cycles (vs. padding every core to
    max(count) as pure expert-parallelism would).
  - Each core runs a dense FFN (gelu(x@W1.T+b1)@W2.T+b2) over both
    segments in one SPMD Bass program; host scatters y back with the
    combine weights.

Device kernel layout (per core):
  Segment A tiles (<=512 tokens each):
    FFN1: psum[inter128, tok] += W1T[k,m].T @ xT[k, tok];  h = gelu(+b1)
    FFN2: psum[hid128, tok]  += W2T[k,m].T @ h[k, tok];    y = psum + b2
  The LAST A tile's FFN2 runs k-OUTER (all 8 m-psums live at once) so
  each w2A[k] retires early and w2B[k] can stream into its buffer during
  that tile -- segment B's weights (16 MB) are fully resident by the time
  segment B's matmuls start, with no PE stall.
  Weights held in SBUF as bf16; x streams on GpSimd SWDGE queues.
"""

import sys
import types

import numpy as np
import ml_dtypes

import concourse.bass as bass
import concourse.tile as tile
from concourse import mybir
from concourse.bass_utils import run_bass_kernel_spmd
from bass_rust import ScopedClock, VectorClock


def _ensure_axon_hooks():
    """run_bass_kernel_spmd(trace=True) under axon imports antenv.axon_hooks,
    which this image's antenv lacks.  Register an equivalent module backed by
    trn_agent_boot's ctypes NTFF hook so tracing works (and trace=False paths
    are unaffected)."""
    try:
        import antenv.axon_hooks  # noqa: F401
        return
    except ImportError:
        pass
    hook = None
    try:
        from trn_agent_boot.trn_boot import _ntff_profile_via_ctypes
        hook = _ntff_profile_via_ctypes("/opt/axon/libaxon_pjrt.so")
    except Exception:
        hook = None
    mod = types.ModuleType("antenv.axon_hooks")
    _state = {"hook": hook}
    mod.get_axon_ntff_profile_hook = lambda: _state["hook"]
    mod.set_axon_ntff_profile_hook = lambda h: _state.__setitem__("hook", h)
    sys.modules["antenv.axon_hooks"] = mod
    try:
        import antenv
        antenv.axon_hooks = mod
    except ImportError:
        pass


_ensure_axon_hooks()

H = 1024          # hidden
I = 4096          # intermediate
E = 8             # experts
NCORES = 8
KH = H // 128     # 8  k-tiles over hidden
KI = I // 128     # 32 k-tiles over inter
BF16 = mybir.dt.bfloat16
F32 = mybir.dt.float32
LS_FLOOR = 135    # effective min cycles/matmul (LDWEIGHTS bound), measured


class _TC(tile.TileContext):
    """TileContext whose tail drain splits its sem waits across SP nops.

    The walrus pinned in this container rejects a Drain instruction carrying
    more than a couple of sync waits ("Too many sync wait commands",
    CoreV3GenImpl.cpp:104).  Emit one wait-carrier nop per logical processor
    instead, then a waitless drain.
    """

    def _drain_and_barrier(self, tick_clock, wait_clock):
        nc = self.nc
        gc = tick_clock.global_clock
        ticks = eval(repr(gc).replace("VectorClock(", "").rstrip(")"))
        for i, t in enumerate(ticks):
            if t > 0:
                partial = [0] * len(ticks)
                partial[i] = t
                carrier = nc.sync.nop(nofuse=True, hint=f"drain_wait_{i}")
                wait_clock.add_sem_waits(
                    carrier.ins, ScopedClock({None: VectorClock(partial)})
                )
        nc.sync.drain()
        nc.all_engine_barrier()
        assert self.sems is not None
        popped = nc._tile_sem_poison_stack.pop()
        assert popped is self._sem_poison
        nc.clear_and_free_semaphores(list(self.sems.allocated().values()))
        nc.all_engine_barrier()


def _split_waits(nc, maxw=1):
    """The pinned walrus rejects instructions carrying more than one
    embedded sync wait ("Too many sync wait commands").  Hoist excess waits
    onto freshly inserted same-engine nops placed directly before the
    instruction — the engine sequencer executes them in order, so the
    semantics are identical."""
    for fn in nc.m.functions:
        for bb in fn.blocks:
            new = []
            changed = False
            for inst in bb.instructions:
                si = inst.sync_info
                waits = list(si.on_wait) if si is not None else []
                if len(waits) > maxw:
                    changed = True
                    n_extra = len(waits) - maxw
                    for i in range(0, n_extra, maxw):
                        nop = mybir.InstNoOp(
                            name=nc.get_next_instruction_name(),
                            engine=inst.engine,
                            sync_info=mybir.SyncInfo(
                                on_wait=waits[i:i + maxw], on_update=[]
                            ),
                            bass_nofuse=True,
                        )
                        nc.register_instruction(nop, overwrite=True)
                        new.append(nop)
                    si.on_wait = waits[n_extra:]
                new.append(inst)
            if changed:
                bb.instructions = new


def _tiles_for(A):
    """Token tiles for a segment: remainder FIRST, 512s after (the last
    tile must be wide -- it is the DMA window for the B-weight swap)."""
    if A <= 512:
        return [A]
    rem = A % 512
    tiles = ([rem] if rem else []) + [512] * (A // 512)
    return tiles


def _seg_cost(A):
    return sum(max(tw, LS_FLOOR) for tw in _tiles_for(A)) if A > 0 else 0


def _plan(counts):
    """Pick (A, B) minimizing per-core PE cost such that the expert counts
    pack into 8 A-slots (one per expert) + 8 B-slots (spill pieces)."""
    maxc = max(counts)
    best = (_seg_cost(maxc), maxc, 0)
    for B in range(32, 513, 4):
        lo, hi = 1, maxc
        while lo < hi:
            mid = (lo + hi) // 2
            need = sum(-(-max(0, n - mid) // B) for n in counts)
            if need <= NCORES:
                hi = mid
            else:
                lo = mid + 1
        A = lo
        cost = _seg_cost(A) + max(B, LS_FLOOR)
        if cost < best[0]:
            best = (cost, A, B)
    return best[1], best[2]


def _build(A, B):
    """Two-segment dense FFN; one SPMD program for all cores."""
    nc = bass.Bass()
    # 3D dram views ([k, 128, cols]) let one DMA descriptor cover all
    # k-tiles of a column phase (merged loads cut engine issue latency).
    xta = nc.declare_dram_parameter("xta", [KH, 128, A], BF16, isOutput=False)
    w1a = nc.declare_dram_parameter("w1a", [KH, 128, I], BF16, isOutput=False)
    w2a = nc.declare_dram_parameter("w2a", [KI, 128, H], BF16, isOutput=False)
    b1a = nc.declare_dram_parameter("b1a", [128, KI], F32, isOutput=False)
    b2a = nc.declare_dram_parameter("b2a", [128, KH], F32, isOutput=False)
    yta = nc.declare_dram_parameter("yta", [H, A], F32, isOutput=True)
    if B:
        xtb = nc.declare_dram_parameter("xtb", [KH, 128, B], BF16, isOutput=False)
        w1b = nc.declare_dram_parameter("w1b", [KH, 128, I], BF16, isOutput=False)
        w2b = nc.declare_dram_parameter("w2b", [KI, 128, H], BF16, isOutput=False)
        b1b = nc.declare_dram_parameter("b1b", [128, KI], F32, isOutput=False)
        b2b = nc.declare_dram_parameter("b2b", [128, KH], F32, isOutput=False)
        ytb = nc.declare_dram_parameter("ytb", [H, B], F32, isOutput=True)

    tiles = _tiles_for(A)
    t0 = tiles[0]
    sub0 = min(256, t0)          # first psum groups fire after only sub0 tokens

    with _TC(nc) as tc:
        with (
            tc.tile_pool(name="w1p", bufs=1) as w1pool,
            tc.tile_pool(name="w2p", bufs=1) as w2pool,
            tc.tile_pool(name="bias", bufs=1) as bpool,
            tc.tile_pool(name="x", bufs=3) as xpool,
            tc.tile_pool(name="h", bufs=1) as hpool,
            tc.tile_pool(name="o", bufs=4) as opool,
            tc.tile_pool(name="ps", bufs=8, space="PSUM") as pspool,
        ):
            def kpc(dram3d, lo, hi):
                # [k,128,c] slice iterated partition-major to match SBUF
                return dram3d[:, :, lo:hi].rearrange("k p c -> p k c")

            # ---- load issue: biases tiny, on gpsimd ----
            b1s = bpool.tile([128, KI], F32, tag="b1")
            nc.gpsimd.dma_start(b1s[:], b1a[:])
            b2s = bpool.tile([128, KH], F32, tag="b2")
            nc.gpsimd.dma_start(b2s[:], b2a[:])
            if B:
                b1sB = bpool.tile([128, KI], F32, tag="b1B")
                nc.gpsimd.dma_start(b1sB[:], b1b[:])
                b2sB = bpool.tile([128, KH], F32, tag="b2B")
                nc.gpsimd.dma_start(b2sB[:], b2b[:])

            # First sub-tile of x on the (otherwise idle) scalar HWDGE
            # queue, in parallel with w1 phase 0 on sync: the first matmul
            # needs both.
            xs0 = xpool.tile([128, KH * 512], BF16, tag="xt", name="xs0")
            xs0v = xs0[:].rearrange("p (k c) -> p k c", k=KH)
            nc.scalar.dma_start(xs0v[:, :, 0:sub0], kpc(xta, 0, sub0))

            # W1 (one [128, KH*I] tile): column-phased merged loads.
            # Phase 0 small (128 cols) so the first psum group unblocks
            # fast; then coarse phases (bigger rows = full DMA rate).
            w1s = w1pool.tile([128, KH * I], BF16, tag="w1", name="w1s")
            w1v = w1s[:].rearrange("p (k c) -> p k c", k=KH)
            bounds = [0, 128, 1120, 2112, 3104, 4096]
            for lo, hi in zip(bounds[:-1], bounds[1:]):
                nc.sync.dma_start(w1v[:, :, lo:hi], kpc(w1a, lo, hi))

            # rest of x tile 0 on gpsimd SWDGE (needed only ~27us in)
            if t0 > sub0:
                for k in range(KH):
                    nc.gpsimd.dma_start(
                        xs0[:, k * 512 + sub0:k * 512 + t0],
                        xta[k, :, sub0:t0],
                    )

            # W2 per-k tiles (k-granular so segment B's w2 can stream in
            # as each w2a[k] retires during the last A tile's FFN2).
            w2s = []
            for k in range(KI):
                w = w2pool.tile([128, H], BF16, tag=f"w2_{k}")
                nc.sync.dma_start(w[:], w2a[k, :, :])
                w2s.append(w)

            # ---- segment A compute ----
            xtiles = [xs0]
            off = t0
            for tw in tiles[1:]:
                xs = xpool.tile([128, KH * 512], BF16, tag="xt")
                for k in range(KH):
                    nc.gpsimd.dma_start(
                        xs[:, k * 512:k * 512 + tw],
                        xta[k, :, off:off + tw],
                    )
                xtiles.append(xs)
                off += tw
            if B:
                xsB = xpool.tile([128, KH * B], BF16, tag="xb")
                for k in range(KH):
                    nc.gpsimd.dma_start(xsB[:, k * B:(k + 1) * B], xtb[k, :, :])

            def ffn1(xs, ht, tw, subs):
                for m in range(KI):
                    clo = 0
                    for sw in subs:
                        ps = pspool.tile([128, 512], F32, tag="ps")
                        for k in range(KH):
                            nc.tensor.matmul(
                                ps[:, 0:sw],
                                w1s[:, k * I + m * 128:k * I + (m + 1) * 128],
                                xs[:, k * 512 + clo:k * 512 + clo + sw],
                                start=(k == 0),
                                stop=(k == KH - 1),
                            )
                        nc.scalar.activation(
                            ht[:, m * 512 + clo:m * 512 + clo + sw],
                            ps[:, 0:sw],
                            mybir.ActivationFunctionType.Gelu,
                            bias=b1s[:, m:m + 1],
                        )
                        clo += sw

            def ffn2_store(ps, tw, m, off):
                ot = opool.tile([128, 512], F32, tag="o")
                nc.vector.tensor_scalar_add(ot[:, 0:tw], ps[:, 0:tw], b2s[:, m:m + 1])
                nc.scalar.dma_start(yta[m * 128:(m + 1) * 128, off:off + tw], ot[:, 0:tw])

            off = 0
            for ti, tw in enumerate(tiles):
                xs = xtiles[ti]
                ht = hpool.tile([128, KI * 512], BF16, tag="h")
                subs = [sub0, t0 - sub0] if (ti == 0 and t0 > sub0) else [tw]
                ffn1(xs, ht, tw, subs)
                last = ti == len(tiles) - 1
                if not last:
                    for m in range(KH):
                        ps = pspool.tile([128, 512], F32, tag="ps")
                        for k in range(KI):
                            nc.tensor.matmul(
                                ps[:, 0:tw],
                                w2s[k][:, m * 128:(m + 1) * 128],
                                ht[:, k * 512:k * 512 + tw],
                                start=(k == 0),
                                stop=(k == KI - 1),
                            )
                        ffn2_store(ps, tw, m, off)
                else:
                    # k-OUTER: w2s[k]'s last read is phase k, so its buffer
                    # frees early and w2B[k] streams in behind it.
                    psl = [
                        pspool.tile([128, 512], F32, tag="ps", name=f"psl{m}")
                        for m in range(KH)
                    ]
                    for k in range(KI):
                        for m in range(KH):
                            nc.tensor.matmul(
                                psl[m][:, 0:tw],
                                w2s[k][:, m * 128:(m + 1) * 128],
                                ht[:, k * 512:k * 512 + tw],
                                start=(k == 0),
                                stop=(k == KI - 1),
                                skip_group_check=True,
                            )
                    for m in range(KH):
                        ffn2_store(psl[m], tw, m, off)
                off += tw

            # ---- segment B: weights swap into the A buffers ----
            if B:
                # w1B: one merged DMA; WAR waits for segment A's last FFN1
                # read, then streams during the last A tile's FFN2.
                w1sB = w1pool.tile([128, KH * I], BF16, tag="w1", name="w1sB")
                w1vB = w1sB[:].rearrange("p (k c) -> p k c", k=KH)
                nc.sync.dma_start(w1vB[:], kpc(w1b, 0, I))
                w2sB = []
                for k in range(KI):
                    w = w2pool.tile([128, H], BF16, tag=f"w2_{k}")
                    nc.sync.dma_start(w[:], w2b[k, :, :])
                    w2sB.append(w)

                htB = hpool.tile([128, KI * B], BF16, tag="hb")
                for m in range(KI):
                    ps = pspool.tile([128, 512], F32, tag="ps")
                    for k in range(KH):
                        nc.tensor.matmul(
                            ps[:, 0:B],
                            w1sB[:, k * I + m * 128:k * I + (m + 1) * 128],
                            xsB[:, k * B:(k + 1) * B],
                            start=(k == 0),
                            stop=(k == KH - 1),
                        )
                    nc.scalar.activation(
                        htB[:, m * B:(m + 1) * B],
                        ps[:, 0:B],
                        mybir.ActivationFunctionType.Gelu,
                        bias=b1sB[:, m:m + 1],
                    )
                for m in range(KH):
                    ps = pspool.tile([128, 512], F32, tag="ps")
                    for k in range(KI):
                        nc.tensor.matmul(
                            ps[:, 0:B],
                            w2sB[k][:, m * 128:(m + 1) * 128],
                            htB[:, k * B:(k + 1) * B],
                            start=(k == 0),
                            stop=(k == KI - 1),
                        )
                    ot = opool.tile([128, 512], F32, tag="o")
                    nc.vector.tensor_scalar_add(
                        ot[:, 0:B], ps[:, 0:B], b2sB[:, m:m + 1]
                    )
                    nc.scalar.dma_start(ytb[m * 128:(m + 1) * 128, :], ot[:, 0:B])
    _split_waits(nc)
    return nc


def _route(x, gate_w):
    """Host gate: top-2 of 8 logits + softmax over the selected pair."""
    logits = x @ gate_w.T                         # [T, E] f32
    T = logits.shape[0]
    rows = np.arange(T)
    i1 = np.argmax(logits, axis=1)
    v1 = logits[rows, i1]
    masked = logits.copy()
    masked[rows, i1] = -np.inf
    i2 = np.argmax(masked, axis=1)
    v2 = masked[rows, i2]
    # softmax over (v1, v2) with v1 >= v2
    e2 = np.exp(v2 - v1)
    w1 = 1.0 / (1.0 + e2)
    w2 = 1.0 - w1
    return i1, i2, w1.astype(np.float32), w2.astype(np.float32)


def _weight_maps(W1, b1, W2, b2, e):
    return {
        "w1": np.ascontiguousarray(
            W1[e].astype(ml_dtypes.bfloat16).T
        ).reshape(KH, 128, I),
        "w2": np.ascontiguousarray(
            W2[e].astype(ml_dtypes.bfloat16).T
        ).reshape(KI, 128, H),
        "b1": np.ascontiguousarray(b1[e].reshape(KI, 128).T),
        "b2": np.ascontiguousarray(b2[e].reshape(KH, 128).T),
    }


def _xmap(x, toks, C):
    xe = np.zeros((C, H), dtype=ml_dtypes.bfloat16)
    xe[: len(toks)] = x[toks].astype(ml_dtypes.bfloat16)
    return np.ascontiguousarray(xe.T).reshape(KH, 128, C)


def _run(inputs, trace=False):
    hidden_states = np.asarray(inputs["hidden_states"], dtype=np.float32)
    gate_w = np.asarray(inputs["gate_w"], dtype=np.float32)
    W1 = np.asarray(inputs["W1"], dtype=np.float32)
    b1 = np.asarray(inputs["b1"], dtype=np.float32)
    W2 = np.asarray(inputs["W2"], dtype=np.float32)
    b2 = np.asarray(inputs["b2"], dtype=np.float32)

    B_, S, _ = hidden_states.shape
    T = B_ * S
    x = np.ascontiguousarray(hidden_states.reshape(T, H))

    i1, i2, w1, w2 = _route(x, gate_w)
    toks = [np.flatnonzero((i1 == e) | (i2 == e)) for e in range(E)]
    cnts = [len(t) for t in toks]

    A, B = _plan(cnts)

    # A-slot e on core e; spill pieces round-robin over cores' B-slots.
    a_slots = [(e, toks[e][:min(cnts[e], A)]) for e in range(E)]
    pieces = []
    for e in range(E):
        spill = toks[e][A:]
        for s in range(0, len(spill), max(B, 1)):
            pieces.append((e, spill[s:s + B]))
    assert len(pieces) <= NCORES
    b_slots = [pieces[i] if i < len(pieces) else None for i in range(NCORES)]

    nc = _build(A, B)

    wcache = {}

    def wmap(e):
        if e not in wcache:
            wcache[e] = _weight_maps(W1, b1, W2, b2, e)
        return wcache[e]

    in_maps = []
    for c in range(NCORES):
        ea, ta = a_slots[c]
        wa = wmap(ea)
        m = {
            "xta": _xmap(x, ta, A),
            "w1a": wa["w1"], "w2a": wa["w2"],
            "b1a": wa["b1"], "b2a": wa["b2"],
        }
        if B:
            eb, tb = b_slots[c] if b_slots[c] is not None else (ea, [])
            wb = wmap(eb)
            m.update({
                "xtb": _xmap(x, tb, B),
                "w1b": wb["w1"], "w2b": wb["w2"],
                "b1b": wb["b1"], "b2b": wb["b2"],
            })
        in_maps.append(m)

    res = run_bass_kernel_spmd(
        nc, in_maps, core_ids=list(range(NCORES)), trace=trace
    )

    out = np.zeros((T, H), dtype=np.float32)

    def scatter(te, y):
        we = np.where(i1[te] == e_, w1[te], w2[te])
        out[te] += we[:, None] * y

    for c in range(NCORES):
        e_, ta = a_slots[c]
        ya = res.results[c]["yta"][:, : len(ta)].T
        scatter(ta, ya)
        if B and b_slots[c] is not None:
            e_, tb = b_slots[c]
            if len(tb):
                yb = res.results[c]["ytb"][:, : len(tb)].T
                scatter(tb, yb)
    return out.reshape(B_, S, H), res


def kernel(**inputs):
    out, _ = _run(inputs, trace=False)
    return out
